# revision 35
# baseline (speedup 1.0000x reference)
"""AdaptiveGraphLayer Trainium2 kernel (8 NeuronCores, data-parallel over B).

Host precomputes the (x-independent) masked-softmax adjacency, the per-batch
gate (tiny MLP on the temporal-mean context), and algebraically fused weights:

    out = g*(A@x)@Wc1^T + ((g*(A@x)@Wmul^T + b_mul) * x) @ Wo2^T + bc + x
    Wc1 = Wout[:, :D] @ Wadd,  bc = b_out + Wout[:, :D] @ b_add
    A   = diag(gate_b) @ softmax(mask(emb1@emb2^T))         (per batch b)
    residual + b_mul term folded into R = I + (Wo2 * b_mul[None, :])^T

LayerNorm centering is folded into the weights: every output-side weight is
post-multiplied by the centering matrix C = I - 11^T/D, so the device
produces y_c = (x + out) @ C = y - mean(y) directly.  The device also emits
sum(y_c^2) per row; the host applies z = y_c * rsqrt(ssq/D + eps) (* gamma
+ beta), which is exact LayerNorm.

Each core processes one batch element: x shard [T=64, N=256, D=128].
Device dataflow per 2-timestep block (bf16 compute, f32 PSUM accumulation),
software-pipelined 3 deep so TensorE/ScalarE/VectorE stages of consecutive
blocks overlap:
  aggrT[d,n] = sum_k x[t]_chunk[k].T @ A^T_chunk[k]        (TensorE)
  copy aggrT -> SBUF bf16                                  (ScalarE)
  m1T [o,n]  = Wmul^T.T @ aggrT                            (TensorE)
  mulT[d,n]  = m1T * xT                                    (VectorE)
  y_c[n,o]   = aggr@Wc1C + mul@Wo2C + x@RC  (3 accumulating matmuls,
               activations as stationary lhsT -> natural [n,d] output)
  copy y_c -> SBUF bf16                                    (ScalarE)
  ssq[row]   = sum_d y_c^2   (tensor_tensor_reduce)        (VectorE)
"""

import numpy as np
import ml_dtypes

BF16 = ml_dtypes.bfloat16

B, T, N, D = 8, 64, 256, 128
P = 128          # partitions / n-chunk size
G = N // P       # n-chunks per timestep (2)
TB = 4           # timesteps per PSUM block
NBLK = T // TB
THRESH = 0.01
NCORES = 8

_CACHE = {}


def _build(bres_nonzero: bool):
    from contextlib import ExitStack

    import concourse.tile as tile
    import concourse.mybir as mybir
    from concourse import bacc

    dt = mybir.dt
    Alu = mybir.AluOpType

    nc = bacc.Bacc("TRN2", target_bir_lowering=False, debug=False,
                   num_devices=NCORES)

    adjt = nc.declare_dram_parameter("adjt", [P, G, N], dt.bfloat16, False)
    wc1t = nc.declare_dram_parameter("wc1t", [P, D], dt.bfloat16, False)
    wmult = nc.declare_dram_parameter("wmult", [P, D], dt.bfloat16, False)
    wo2t = nc.declare_dram_parameter("wo2t", [P, D], dt.bfloat16, False)
    if bres_nonzero:
        rres = nc.declare_dram_parameter("rres", [P, D], dt.bfloat16, False)
    x2 = nc.declare_dram_parameter("x2", [P, T, G, D], dt.bfloat16, False)
    x2t = nc.declare_dram_parameter("x2t", [P, T, N], dt.bfloat16, False)
    out = nc.declare_dram_parameter("out", [P, T, G, D], dt.bfloat16, True)

    with tile.TileContext(nc) as tc, ExitStack() as ctx:
        consts = ctx.enter_context(tc.tile_pool(name="consts", bufs=1))
        xpool = ctx.enter_context(tc.tile_pool(name="x", bufs=1))
        work = ctx.enter_context(tc.tile_pool(name="work", bufs=5))
        ypool = ctx.enter_context(tc.tile_pool(name="y", bufs=4))
        pp = ctx.enter_context(tc.tile_pool(name="pp", bufs=2, space="PSUM"))
        py = ctx.enter_context(tc.tile_pool(name="py", bufs=2, space="PSUM"))

        QB = 2  # blocks per x-load DMA
        NQ = NBLK // QB
        adjt_sb = consts.tile([P, G, N], dt.bfloat16, tag="adjt")
        wc1t_sb = consts.tile([P, D], dt.bfloat16, tag="wc1t")
        wmult_sb = consts.tile([P, D], dt.bfloat16, tag="wmult")
        wo2t_sb = consts.tile([P, D], dt.bfloat16, tag="wo2t")
        rres_sb = (consts.tile([P, D], dt.bfloat16, tag="rres", name="rres_sb")
                   if bres_nonzero else None)
        xn = [xpool.tile([P, QB * TB, G, D], dt.bfloat16, tag=f"xn{q}",
                         name=f"xn{q}") for q in range(NQ)]
        xt = [xpool.tile([P, QB * TB, N], dt.bfloat16, tag=f"xt{q}",
                         name=f"xt{q}") for q in range(NQ)]

        # Startup-latency-critical transfers go first: adjacency, then block
        # 0/1's x split per block, then the small weights.  Remaining quads
        # are big transfers on the scalar sequencer (parallel issue stream).
        nc.sync.dma_start(out=adjt_sb[:], in_=adjt[:])
        for h in range(QB):
            t0 = h * TB
            nc.sync.dma_start(out=xn[0][:, t0:t0 + TB, :, :],
                              in_=x2[:, t0:t0 + TB, :, :])
            nc.sync.dma_start(out=xt[0][:, t0:t0 + TB, :],
                              in_=x2t[:, t0:t0 + TB, :])
        nc.sync.dma_start(out=wc1t_sb[:], in_=wc1t[:])
        nc.sync.dma_start(out=wmult_sb[:], in_=wmult[:])
        nc.sync.dma_start(out=wo2t_sb[:], in_=wo2t[:])
        if bres_nonzero:
            nc.sync.dma_start(out=rres_sb[:], in_=rres[:])
        for q in range(1, NQ):
            t0 = q * QB * TB
            nc.scalar.dma_start(out=xn[q][:], in_=x2[:, t0:t0 + QB * TB, :, :])
            nc.scalar.dma_start(out=xt[q][:], in_=x2t[:, t0:t0 + QB * TB, :])

        def xn_sl(b, ti):
            return xn[b // QB][:, (b % QB) * TB + ti, :, :]

        def xt_sl(b, ti):
            return xt[b // QB][:, (b % QB) * TB + ti, :]

        # HAM warm-up: ~4us of dummy matmuls on the adjacency tile so the
        # PE clock is at 8/8 when the first real block starts (and the PE
        # has work while the first x tiles are still in flight).
        warm = pp.tile([P, 512], dt.float32, tag="pp", name="warm")
        for w in range(16):
            nc.tensor.matmul(warm[:, :256], adjt_sb[:, 0, :128],
                             adjt_sb[:, 1, :], start=True, stop=True)
        agg_tiles = {}
        mul_tiles = {}

        pp_tiles = {}

        def stage_a(b):
            # aggrT = (A_g @ x[t]).T for TB timesteps -> SBUF bf16
            pa_t = pp.tile([P, TB * N], dt.float32, tag="pp", name="pa_t")
            pp_tiles[b] = pa_t
            for ti in range(TB):
                for k in range(G):
                    nc.tensor.matmul(
                        pa_t[:, ti * N:(ti + 1) * N],
                        xn_sl(b, ti)[:, k, :],
                        adjt_sb[:, k, :],
                        start=(k == 0), stop=(k == G - 1),
                    )
            agg_sb = work.tile([P, TB, N], dt.bfloat16, tag="agg",
                               name="agg_sb")
            nc.vector.tensor_copy(
                out=agg_sb[:],
                in_=pa_t[:].rearrange("p (t n) -> p t n", t=TB),
            )
            agg_tiles[b] = agg_sb

        def stage_m(b):
            # m1T = Wmul @ aggrT ; mulT = m1T * xT -> SBUF bf16
            agg_sb = agg_tiles[b]
            pm_t = pp_tiles.pop(b)
            for h in range(TB * N // 512):
                nc.tensor.matmul(
                    pm_t[:, h * 512:(h + 1) * 512],
                    wmult_sb[:],
                    agg_sb[:].rearrange("p t n -> p (t n)")[:, h * 512:(h + 1) * 512],
                    start=True, stop=True,
                )
            mul_sb = work.tile([P, TB, N], dt.bfloat16, tag="mul",
                               name="mul_sb")
            nc.vector.tensor_tensor(
                out=mul_sb[:],
                in0=pm_t[:].rearrange("p (t n) -> p t n", t=TB),
                in1=xt[b // QB][:, (b % QB) * TB:(b % QB) * TB + TB, :],
                op=Alu.mult,
            )
            mul_tiles[b] = mul_sb

        def stage_s(b):
            # s = aggr@Wc1^T + mul@Wo2^T (+ x@Rres^T when b_mul != 0);
            # residual + LayerNorm are applied on the host.
            agg_sb = agg_tiles.pop(b)
            mul_sb = mul_tiles.pop(b)
            py_t = py.tile([P, TB * G * D], dt.float32, tag="py", name="py_t")
            for ti in range(TB):
                for c in range(G):
                    o = py_t[:, (ti * G + c) * D:(ti * G + c + 1) * D]
                    nc.tensor.matmul(
                        o, agg_sb[:, ti, c * D:(c + 1) * D], wc1t_sb[:],
                        start=True, stop=False)
                    nc.tensor.matmul(
                        o, mul_sb[:, ti, c * D:(c + 1) * D], wo2t_sb[:],
                        start=False, stop=not bres_nonzero)
                    if bres_nonzero:
                        nc.tensor.matmul(
                            o, xt_sl(b, ti)[:, c * D:(c + 1) * D], rres_sb[:],
                            start=False, stop=True)
            y_sb = ypool.tile([P, TB, G, D], dt.bfloat16, tag="ysb",
                              name="y_sb")
            nc.scalar.copy(
                out=y_sb[:],
                in_=py_t[:].rearrange("p (t g d) -> p t g d", t=TB, g=G),
            )
            t0 = b * TB
            nc.sync.dma_start(out=out[:, t0:t0 + TB, :, :], in_=y_sb[:])

        # 4-deep software pipeline: M(b-1) || A(b) || S(b-3)
        for i in range(NBLK + 3):
            if 1 <= i < NBLK + 1:
                stage_m(i - 1)
            if i < NBLK:
                stage_a(i)
            if i >= 3:
                stage_s(i - 3)

    nc.compile()
    return nc


def _softmax(x, axis=-1):
    m = np.max(x, axis=axis, keepdims=True)
    e = np.exp(x - m)
    return e / np.sum(e, axis=axis, keepdims=True)


TRACE = False


def _ensure_profile_hook():
    """Register the NTFF profile hook if the image's antenv lacks it."""
    import sys
    import types
    try:
        from antenv import axon_hooks  # noqa: F401
        return
    except ImportError:
        pass
    try:
        from trn_agent_boot.trn_boot import _ntff_profile_via_ctypes
        hook = _ntff_profile_via_ctypes("/opt/axon/libaxon_pjrt.so")
    except Exception:
        hook = None
    mod = types.ModuleType("antenv.axon_hooks")
    mod.get_axon_ntff_profile_hook = lambda: hook
    mod.set_axon_ntff_profile_hook = lambda h: None
    sys.modules["antenv.axon_hooks"] = mod


LDW_OPT = False


def _patch_ldw_opt():
    import concourse.bass_utils as bu
    if getattr(bu, "_ldw_patched", False):
        return
    orig = bu.run_command

    def patched(argv, **kw):
        argv = ["--enable-ldw-opt=true" if a == "--enable-ldw-opt=false" else a
                for a in argv]
        return orig(argv, **kw)

    bu.run_command = patched
    bu._ldw_patched = True


def kernel(x, emb1, emb2, W_add, b_add, W_mul, b_mul, Wa1, ba1, Wa2, ba2,
           W_out, b_out, gamma, beta):
    import concourse.bass_utils as bass_utils
    from concourse.bass_utils import run_bass_kernel_spmd
    if LDW_OPT:
        _patch_ldw_opt()
    if TRACE:
        _ensure_profile_hook()
        bass_utils.upload_artifacts = lambda tmpdir: tmpdir

    x = np.asarray(x, np.float32)
    emb1 = np.asarray(emb1, np.float32)
    emb2 = np.asarray(emb2, np.float32)
    W_add = np.asarray(W_add, np.float32)
    b_add = np.asarray(b_add, np.float32)
    W_mul = np.asarray(W_mul, np.float32)
    b_mul = np.asarray(b_mul, np.float32)
    Wa1 = np.asarray(Wa1, np.float32)
    ba1 = np.asarray(ba1, np.float32)
    Wa2 = np.asarray(Wa2, np.float32)
    ba2 = np.asarray(ba2, np.float32)
    W_out = np.asarray(W_out, np.float32)
    b_out = np.asarray(b_out, np.float32)
    gamma = np.asarray(gamma, np.float32)
    beta = np.asarray(beta, np.float32)

    # ---- host: shared adjacency + per-batch gate ----
    raw = emb1 @ emb2.T
    masked = np.where(raw > THRESH, raw, np.float32(-1e9))
    adj = _softmax(masked, -1)                        # [N, N]
    ctx_m = x.mean(axis=1)                            # [B, N, D]
    h = np.maximum(ctx_m @ Wa1.T + ba1, 0.0)
    gate = 1.0 / (1.0 + np.exp(-(h @ Wa2.T + ba2)))   # [B, N, 1]
    gate = gate[..., 0]                               # [B, N]

    W_out1 = W_out[:, :D]
    W_out2 = W_out[:, D:]
    Wc1 = W_out1 @ W_add                              # [o, d]
    bc = b_out + W_out1 @ b_add
    bres_nonzero = bool(np.any(b_mul != 0.0))

    key = bres_nonzero
    if key not in _CACHE:
        _CACHE[key] = _build(bres_nonzero)
    nc = _CACHE[key]

    wc1t_np = np.ascontiguousarray(Wc1.T).astype(BF16)
    wmult_np = np.ascontiguousarray(W_mul.T).astype(BF16)
    wo2t_np = np.ascontiguousarray(W_out2.T).astype(BF16)
    rres_np = np.ascontiguousarray((W_out2 * b_mul[None, :]).T).astype(BF16)

    in_maps = []
    for b in range(NCORES):
        A_b = adj * gate[b][:, None]                  # [n, n']
        adjt_np = np.ascontiguousarray(
            A_b.T.reshape(G, P, N).transpose(1, 0, 2)).astype(BF16)
        xb = x[b]                                     # [T, N, D]
        x2_np = np.ascontiguousarray(
            xb.reshape(T, G, P, D).transpose(2, 0, 1, 3)).astype(BF16)
        x2t_np = np.ascontiguousarray(
            xb.transpose(2, 0, 1)).astype(BF16)       # [D, T, N]
        m = {
            "adjt": adjt_np, "wc1t": wc1t_np, "wmult": wmult_np,
            "wo2t": wo2t_np, "x2": x2_np, "x2t": x2t_np,
        }
        if bres_nonzero:
            m["rres"] = rres_np
        in_maps.append(m)

    res = run_bass_kernel_spmd(nc, in_maps, core_ids=list(range(NCORES)),
                               trace=TRACE)
    import kernel as _self
    _self.LAST_RESULT = res

    outs = np.empty((B, T, N, D), np.float32)
    for b in range(NCORES):
        s = np.asarray(res.results[b]["out"]).astype(np.float32)
        # s: [P, T, G, D] matmul update; y = x + s + bc, then LayerNorm.
        y = s.transpose(1, 2, 0, 3).reshape(T, N, D) + x[b] + bc
        mean = y.mean(-1, keepdims=True)
        var = y.var(-1, keepdims=True)
        outs[b] = (y - mean) / np.sqrt(var + 1e-5)

    if np.any(gamma != 1.0) or np.any(beta != 0.0):
        outs = outs * gamma + beta
    return outs


LAST_RESULT = None
